# revision 1
# baseline (speedup 1.0000x reference)
"""Causal self-attention (B=4, T=2048, C=1024, H=16) on 8 trn2 NeuronCores.

Sharding: tensor-parallel over heads. Core c owns heads (2c, 2c+1).
Each core computes QKV projection for its 2 heads (full x), causal
attention for its (4 batches x 2 heads), and a partial output projection
with its 128 rows of W_proj. Host sums the 8 partial outputs + b_proj.

Device-side layout choices:
  - x is transposed on load (DMA xbar transpose, bf16) to x_T [c_in, t]
  - Q,K are produced transposed: [c_out(=2*64), t] so the scores matmul
    contracts d on partitions; heads live on partition halves 0:64/64:128
    which row-packs the two heads' score matmuls in the PE array.
  - scores S_T [k, q] per 128-k-block; exp on ACT (scale=1/8 folded in),
    causal handled by block skipping + a static triangular 0/1 mask.
  - softmax sums via N=1 matmuls (exp_block.T @ ones) accumulated as
    columns of a per-batch PSUM tile -> reciprocal in column form.
  - AV matmul accumulates y_T [d(2 heads on partition halves), q].
  - normalization: r transposed to rows (PE transpose), broadcast over
    partitions with a tiny selector matmul, multiplied into y on DVE.
  - projection: y_T tiles are lhsT directly; partial out DMAd from PSUM.
"""

import sys

sys.path.insert(0, "/opt/trn_rl_repo")

import numpy as np
import ml_dtypes

B, T, C, H = 4, 2048, 1024, 16
HD = C // H  # 64
BT = B * T  # 8192
NCORES = 8
TCH = 512  # t-chunk
NT = BT // TCH  # 16
NCC = C // 128  # 8 c_in chunks
KB = 128  # k block
QB = 128  # q subblock

_RUNNER = None


def _build_nc():
    import concourse.bacc as bacc
    import concourse.mybir as mybir
    import concourse.tile as tile
    from concourse.masks import make_identity

    f32 = mybir.dt.float32
    bf16 = mybir.dt.bfloat16
    Exp = mybir.ActivationFunctionType.Exp
    Log = mybir.ActivationFunctionType.Ln

    nc = bacc.Bacc(None, target_bir_lowering=False, debug=False)

    xtiles = nc.dram_tensor("xtiles", [NT, 128, NCC, TCH], bf16, kind="ExternalInput")
    wqkv = nc.dram_tensor("wqkv", [C, 384], bf16, kind="ExternalInput")
    bqkv = nc.dram_tensor("bqkv", [384], f32, kind="ExternalInput")
    wproj = nc.dram_tensor("wproj", [128, C], bf16, kind="ExternalInput")
    trimask = nc.dram_tensor("trimask", [128, 128], bf16, kind="ExternalInput")
    out_d = nc.dram_tensor("out", [BT, C], f32, kind="ExternalOutput")
    import os as _os
    _dbg = _os.environ.get("KDBG", "") == "1"
    if _dbg:
        dbg_qt = nc.dram_tensor("dbg_qt", [128, BT], bf16, kind="ExternalOutput")
        dbg_kt = nc.dram_tensor("dbg_kt", [128, BT], bf16, kind="ExternalOutput")
        dbg_v1 = nc.dram_tensor("dbg_v1", [128, BT // 128, 160], bf16, kind="ExternalOutput")
        dbg_yt = nc.dram_tensor("dbg_yt", [128, BT], bf16, kind="ExternalOutput")
        dbg_ex = nc.dram_tensor("dbg_ex", [128, 16, TCH], bf16, kind="ExternalOutput")
        dbg_rt = nc.dram_tensor("dbg_rt", [128, TCH], f32, kind="ExternalOutput")

    with tile.TileContext(nc) as tc:
        with (
            tc.tile_pool(name="const", bufs=1) as const_pool,
            tc.tile_pool(name="big", bufs=1) as big_pool,
            tc.tile_pool(name="sb", bufs=2) as sb_pool,
            tc.tile_pool(name="ps", bufs=1, space="PSUM") as ps_pool,
        ):
            # --- constants ---
            wqkv_sb = const_pool.tile([128, NCC, 384], bf16)
            nc.sync.dma_start(
                wqkv_sb, wqkv.ap().rearrange("(n p) m -> p n m", p=128)
            )
            wproj_sb = const_pool.tile([128, C], bf16)
            nc.sync.dma_start(wproj_sb, wproj.ap())
            bias_sb = const_pool.tile([128, 3], f32)
            nc.sync.dma_start(
                bias_sb, bqkv.ap().rearrange("(n p) -> p n", p=128)
            )
            tri_sb = const_pool.tile([128, 128], bf16)
            nc.sync.dma_start(tri_sb, trimask.ap())
            ones64_sb = const_pool.tile([128, 64], f32)
            nc.vector.memset(ones64_sb, 1.0)
            ident_sb = const_pool.tile([128, 128], bf16)
            make_identity(nc, ident_sb)

            # --- persistent activations ---
            qt_sb = big_pool.tile([128, BT], bf16)  # Q_T [2*64, t]
            kt_sb = big_pool.tile([128, BT], bf16)  # K_T
            # V1 per head [t-block, 80]: cols 0:64 V_h, col 64 ones,
            # cols 65:80 pad (16-element xbar-transpose dest alignment)
            v1h0_sb = big_pool.tile([128, BT // 128, 80], bf16)
            v1h1_sb = big_pool.tile([128, BT // 128, 80], bf16)
            v1_sb = [v1h0_sb, v1h1_sb]
            yt_sb = big_pool.tile([128, BT], bf16)  # y_T [c, t]
            nc.gpsimd.memset(v1h0_sb[:, :, 64:65], 1.0)
            nc.gpsimd.memset(v1h1_sb[:, :, 64:65], 1.0)

            # ---------------- phase 1: QKV projection ----------------
            for tch in range(NT):
                t0 = tch * TCH
                xt = sb_pool.tile([128, NCC, TCH], bf16, tag="xt")
                nc.sync.dma_start(xt, xtiles.ap()[tch])
                for o3 in range(3):  # Q_T, K_T, V_T
                    ps = ps_pool.tile([128, TCH], f32, tag="bank", bufs=2)
                    for cc in range(NCC):
                        nc.tensor.matmul(
                            ps,
                            lhsT=wqkv_sb[:, cc, o3 * 128 : (o3 + 1) * 128],
                            rhs=xt[:, cc, :],
                            start=(cc == 0),
                            stop=(cc == NCC - 1),
                        )
                    if o3 == 0:
                        nc.vector.tensor_scalar_add(
                            qt_sb[:, t0 : t0 + TCH], ps, bias_sb[:, 0:1]
                        )
                    elif o3 == 1:
                        nc.vector.tensor_scalar_add(
                            kt_sb[:, t0 : t0 + TCH], ps, bias_sb[:, 1:2]
                        )
                    else:
                        vtmp = sb_pool.tile([128, TCH], bf16, tag="vtmp")
                        nc.vector.tensor_scalar_add(vtmp, ps, bias_sb[:, 2:3])
                        for j in range(TCH // 128):
                            tb = tch * 4 + j
                            vtp = ps_pool.tile(
                                [128, 128], bf16, tag="bank", bufs=2,
                                name=f"vtp_{tch}_{j}",
                            )
                            nc.tensor.transpose(
                                vtp, vtmp[:, j * 128 : (j + 1) * 128], ident_sb
                            )
                            nc.vector.tensor_copy(
                                v1h0_sb[:, tb, 0:64], vtp[:, 0:64]
                            )
                            nc.vector.tensor_copy(
                                v1h1_sb[:, tb, 0:64], vtp[:, 64:128]
                            )

            # ---------------- phase 2: attention ----------------
            # batch pairs interleaved: while one batch waits on exp, the
            # PE runs the other batch's scores/AV
            for bp in range(B // 2):
              for qc in range(4):
                for b in (2 * bp, 2 * bp + 1):
                    q0 = (b * 4 + qc) * TCH
                    nkb = 4 * qc + 4
                    exps = [
                        sb_pool.tile(
                            [128, 16, TCH],
                            bf16,
                            tag=f"exps{h}",
                            bufs=2,
                            name=f"exps{h}_{b}_{qc}",
                        )
                        for h in (0, 1)
                    ]
                    yps = [
                        ps_pool.tile(
                            [128, TCH],
                            f32,
                            tag="yps",
                            bufs=2,
                            name=f"yps{h}_{b}_{qc}",
                        )
                        for h in (0, 1)
                    ]
                    # scores in groups of 2 k-blocks, heads interleaved so
                    # the PE can overlap the two heads' row-groups
                    for g in range(nkb // 2):
                        for h in (0, 1):
                            hp = h * 64
                            sps = ps_pool.tile(
                                [128, 2 * TCH],
                                f32,
                                tag="sgroup",
                                bufs=2,
                                name=f"sps{h}_{b}_{qc}_{g}",
                            )
                            for i in (0, 1):
                                kb = 2 * g + i
                                k0 = (b * 16 + kb) * 128
                                nc.tensor.matmul(
                                    sps[:, i * TCH : (i + 1) * TCH],
                                    lhsT=kt_sb[hp : hp + 64, k0 : k0 + 128],
                                    rhs=qt_sb[hp : hp + 64, q0 : q0 + TCH],
                                    start=True,
                                    stop=True,
                                )
                            nc.scalar.activation(
                                exps[h][:, 2 * g : 2 * g + 2, :],
                                sps,
                                Exp,
                                scale=0.125,
                            )
                            for i in (0, 1):
                                kb = 2 * g + i
                                j = kb - 4 * qc
                                if j >= 0:
                                    c0 = 128 * j
                                    nc.vector.tensor_mul(
                                        exps[h][:, kb, c0 : c0 + 128],
                                        exps[h][:, kb, c0 : c0 + 128],
                                        tri_sb,
                                    )
                    # AV accumulate (ones column of V1 accumulates the
                    # softmax denominators into row 64)
                    for h in (0, 1):
                        for kb in range(nkb):
                            j = kb - 4 * qc
                            c0 = max(0, 128 * j)
                            nc.tensor.matmul(
                                yps[h][0:65, c0:TCH],
                                lhsT=v1_sb[h][:, b * 16 + kb, 0:65],
                                rhs=exps[h][:, kb, c0:TCH],
                                start=(kb == 0),
                                stop=(kb == nkb - 1),
                            )
                    # evacuate y unnormalized + denominators, freeing the
                    # PSUM banks quickly; normalize SBUF-side below.
                    rt = sb_pool.tile(
                        [128, TCH], f32, tag="rt", name=f"rt_{b}_{qc}"
                    )
                    yu = [
                        sb_pool.tile(
                            [64, TCH], bf16, tag="yu", bufs=4,
                            name=f"yu{h}_{b}_{qc}",
                        )
                        for h in (0, 1)
                    ]
                    nc.vector.tensor_copy(rt[0:1, :], yps[0][64:65, :])
                    nc.vector.tensor_copy(rt[32:33, :], yps[1][64:65, :])
                    nc.vector.tensor_copy(yu[0], yps[0][0:64, :])
                    nc.vector.tensor_copy(yu[1], yps[1][0:64, :])
                    # r = 1/s via fast DVE recip (junk lanes never read)
                    nc.vector.reciprocal_approx_fast(rt[0:33, :], rt[0:33, :])
                    # broadcast r over 64 partitions per head via K=1 matmul
                    for h in (0, 1):
                        hp = h * 64
                        row = 32 * h
                        rb_ps = ps_pool.tile(
                            [64, TCH], f32, tag="bank", bufs=2,
                            name=f"rbps{h}_{b}_{qc}",
                        )
                        nc.tensor.matmul(
                            rb_ps[0:64, :],
                            lhsT=ones64_sb[row : row + 1, :],
                            rhs=rt[row : row + 1, :],
                            start=True,
                            stop=True,
                            tile_position=(row, 0),
                        )
                        rb_sb = sb_pool.tile(
                            [64, TCH], f32, tag="rb", name=f"rbsb{h}_{b}_{qc}"
                        )
                        nc.vector.tensor_copy(rb_sb, rb_ps)
                        nc.vector.tensor_mul(
                            yt_sb[hp : hp + 64, q0 : q0 + TCH],
                            yu[h],
                            rb_sb,
                        )

                    if _dbg and b == 0 and qc == 1:
                        nc.sync.dma_start(dbg_ex.ap(), exps[0])
                        nc.sync.dma_start(dbg_rt.ap(), rt)

              # ------------ phase 3: projection (partial) ------------
              for b in (2 * bp, 2 * bp + 1):
                for tb in range(16):
                    tg = b * 16 + tb
                    oevac = sb_pool.tile([128, C], f32, tag="oevac", bufs=3)
                    for half in range(2):
                        pps = ps_pool.tile([128, TCH], f32, tag="bank", bufs=2)
                        nc.tensor.matmul(
                            pps,
                            lhsT=yt_sb[:, tg * 128 : (tg + 1) * 128],
                            rhs=wproj_sb[:, half * TCH : (half + 1) * TCH],
                            start=True,
                            stop=True,
                        )
                        if half == 0:
                            nc.vector.tensor_copy(
                                oevac[:, half * TCH : (half + 1) * TCH], pps
                            )
                        else:
                            nc.scalar.copy(
                                oevac[:, half * TCH : (half + 1) * TCH], pps
                            )
                    nc.gpsimd.dma_start(
                        out_d.ap()[tg * 128 : (tg + 1) * 128, :], oevac
                    )

            if _dbg:
                nc.sync.dma_start(dbg_qt.ap(), qt_sb)
                nc.sync.dma_start(dbg_kt.ap(), kt_sb)
                nc.sync.dma_start(dbg_v1.ap()[:, :, 0:80], v1h0_sb)
                nc.sync.dma_start(dbg_v1.ap()[:, :, 80:160], v1h1_sb)
                nc.sync.dma_start(dbg_yt.ap(), yt_sb)

    nc.compile()
    return nc


class Runner:
    """Builds the Bass program once and keeps a reusable jitted executor."""

    def __init__(self):
        self.nc = _build_nc()
        self._jit = None
        self._meta = None

    def _build_jit(self):
        import jax
        import numpy as np
        from jax.sharding import Mesh, PartitionSpec
        from jax.experimental.shard_map import shard_map
        import concourse.mybir as mybir
        from concourse import bass2jax

        nc = self.nc
        bass2jax.install_neuronx_cc_hook()

        partition_name = (
            nc.partition_id_tensor.name if nc.partition_id_tensor else None
        )
        in_names, out_names, out_avals = [], [], []
        for alloc in nc.m.functions[0].allocations:
            if not isinstance(alloc, mybir.MemoryLocationSet):
                continue
            name = alloc.memorylocations[0].name
            if alloc.kind == "ExternalInput":
                if name != partition_name:
                    in_names.append(name)
            elif alloc.kind == "ExternalOutput":
                out_names.append(name)
                out_avals.append(
                    jax.core.ShapedArray(
                        tuple(alloc.tensor_shape), mybir.dt.np(alloc.dtype)
                    )
                )
        n_params = len(in_names)
        n_outs = len(out_avals)
        all_in = list(in_names) + list(out_names)
        if partition_name is not None:
            all_in.append(partition_name)

        def _body(*args):
            operands = list(args)
            if partition_name is not None:
                operands.append(bass2jax.partition_id_tensor())
            outs = bass2jax._bass_exec_p.bind(
                *operands,
                out_avals=tuple(out_avals),
                in_names=tuple(all_in),
                out_names=tuple(out_names),
                lowering_input_output_aliases=(),
                sim_require_finite=True,
                sim_require_nnan=True,
                nc=nc,
            )
            return tuple(outs)

        devices = jax.devices()[:NCORES]
        mesh = Mesh(np.asarray(devices), ("core",))
        donate = tuple(range(n_params, n_params + n_outs))
        sharded = jax.jit(
            shard_map(
                _body,
                mesh=mesh,
                in_specs=(PartitionSpec("core"),) * (n_params + n_outs),
                out_specs=(PartitionSpec("core"),) * n_outs,
                check_rep=False,
            ),
            donate_argnums=donate,
            keep_unused=True,
        )
        self._jit = sharded
        self._meta = (in_names, out_names, out_avals)

    def build_timer(self, in_maps, iters):
        """Returns a zero-transfer callable running `iters` chained kernel
        executions on device; inputs are staged on device once."""
        import jax
        import jax.numpy as jnp
        import numpy as np
        from jax.sharding import Mesh, PartitionSpec, NamedSharding
        from jax.experimental.shard_map import shard_map
        import concourse.mybir as mybir
        from concourse import bass2jax

        if self._jit is None:
            self._build_jit()
        nc = self.nc
        in_names, out_names, out_avals = self._meta
        partition_name = (
            nc.partition_id_tensor.name if nc.partition_id_tensor else None
        )
        all_in = list(in_names) + list(out_names)
        if partition_name is not None:
            all_in.append(partition_name)

        n_params = len(in_names)

        def _body(*args):
            ins = list(args[:n_params])
            zeros = list(args[n_params:])
            outs = None
            for _ in range(iters):
                operands = list(ins) + list(zeros)
                if partition_name is not None:
                    operands.append(bass2jax.partition_id_tensor())
                outs = bass2jax._bass_exec_p.bind(
                    *operands,
                    out_avals=tuple(out_avals),
                    in_names=tuple(all_in),
                    out_names=tuple(out_names),
                    lowering_input_output_aliases=(),
                    sim_require_finite=True,
                    sim_require_nnan=True,
                    nc=nc,
                )
            return tuple(outs)

        devices = jax.devices()[:NCORES]
        mesh = Mesh(np.asarray(devices), ("core",))
        spec = NamedSharding(mesh, PartitionSpec("core"))
        fn = jax.jit(
            shard_map(
                _body,
                mesh=mesh,
                in_specs=(PartitionSpec("core"),)
                * (len(in_names) + len(out_names)),
                out_specs=(PartitionSpec("core"),) * len(out_names),
                check_rep=False,
            ),
            keep_unused=True,
        )
        concat_in = [
            jax.device_put(
                np.concatenate([np.asarray(m[name]) for m in in_maps], axis=0),
                spec,
            )
            for name in in_names
        ]
        concat_in += [
            jax.device_put(
                np.zeros((NCORES * a.shape[0], *a.shape[1:]), a.dtype), spec
            )
            for a in out_avals
        ]
        for a in concat_in:
            a.block_until_ready()

        def run():
            outs = fn(*concat_in)
            jax.block_until_ready(outs)
            return outs

        return run

    def execute(self, in_maps):
        """in_maps: list of 8 dicts name->np array. Returns list of out dicts."""
        import numpy as np

        if self._jit is None:
            self._build_jit()
        in_names, out_names, out_avals = self._meta
        concat_in = [
            np.concatenate([np.asarray(m[name]) for m in in_maps], axis=0)
            for name in in_names
        ]
        concat_zeros = [
            np.zeros((NCORES * a.shape[0], *a.shape[1:]), a.dtype)
            for a in out_avals
        ]
        out_arrs = self._jit(*concat_in, *concat_zeros)
        return [
            {
                name: np.asarray(out_arrs[i]).reshape(
                    NCORES, *out_avals[i].shape
                )[c]
                for i, name in enumerate(out_names)
            }
            for c in range(NCORES)
        ]


def make_in_maps(x, W_attn, b_attn, W_proj, b_proj):
    bf16 = ml_dtypes.bfloat16
    xTb = x.reshape(BT, C).T.astype(bf16)  # [C, BT]
    # tiled layout: [tch, p, cc, t] = xT[cc*128+p, tch*512+t], contiguous
    xtiles = np.ascontiguousarray(
        xTb.reshape(NCC, 128, NT, TCH).transpose(2, 1, 0, 3)
    )
    tri = np.tril(np.ones((128, 128), np.float32)).T.astype(bf16)
    # trimask[p, c] = 1 if p <= c  (k index on partitions, q on cols)
    sel2 = np.zeros((2, 128), np.float32)
    sel2[0, :64] = 1.0
    sel2[1, 64:] = 1.0
    in_maps = []
    for c in range(NCORES):
        h0 = 2 * c
        cols = np.r_[h0 * HD : (h0 + 2) * HD]
        wq = W_attn[:, cols]
        wk = W_attn[:, C + cols]
        wv = W_attn[:, 2 * C + cols]
        wqkv = np.concatenate([wq, wk, wv], axis=1).astype(bf16)
        bqkv = np.concatenate(
            [b_attn[cols], b_attn[C + cols], b_attn[2 * C + cols]]
        ).astype(np.float32)
        wproj = np.ascontiguousarray(W_proj[cols, :]).astype(bf16)
        in_maps.append(
            {
                "xtiles": xtiles,
                "wqkv": np.ascontiguousarray(wqkv),
                "bqkv": bqkv,
                "wproj": wproj,
                "trimask": np.ascontiguousarray(tri),
                "sel2": sel2,
            }
        )
    return in_maps


def get_runner():
    global _RUNNER
    if _RUNNER is None:
        _RUNNER = Runner()
    return _RUNNER


def kernel(x, W_attn, b_attn, W_proj, b_proj):
    x = np.asarray(x, dtype=np.float32)
    W_attn = np.asarray(W_attn, dtype=np.float32)
    b_attn = np.asarray(b_attn, dtype=np.float32)
    W_proj = np.asarray(W_proj, dtype=np.float32)
    b_proj = np.asarray(b_proj, dtype=np.float32)
    runner = get_runner()
    in_maps = make_in_maps(x, W_attn, b_attn, W_proj, b_proj)
    results = runner.execute(in_maps)
    total = np.zeros((BT, C), np.float32)
    for r in results:
        total += r["out"]
    total += b_proj[None, :]
    return total.reshape(B, T, C)

